# revision 26
# baseline (speedup 1.0000x reference)
"""Trainium2 Bass kernel for an attention-GRU cell (Bahdanau attention + GRU update).

Computation (per batch row b):
    x   = inputs @ Wi + bi
    xg  = x @ kernel + bias                       (split into x_z, x_r, x_h)
    q   = h_tm1 @ Ua + ba_u
    S   = tanh(context @ Wa + ba_w + q)           [t, U]
    sc  = S @ Va + ba_v                           [t]
    attn = softmax(sc)                            (scores bounded by ||Va||_1 -> no max-sub)
    cv  = sum_t attn * context                    [U]
    cg  = cv @ attention_kernel                   (c_z, c_r, c_h)
    z   = sigmoid(x_z + h@Rz + c_z) ; r = sigmoid(x_r + h@Rr + c_r)
    hb  = tanh(x_h + (r*h)@Rh + c_h)
    h   = z*h_tm1 + (1-z)*hb ; out = h @ Wo + bo

Sharding: batch (64) split across 8 cores, 8 batches/core, weights replicated.
Each core is fully independent (no collectives). Context path runs in fp16
(measured end-to-end error ~3e-4 of out absmax vs the fp32 reference).

Layout strategy per core (measured ~320us on HW; HBM roofline ~100us):
  - context batch slice DMA-loaded with fp32->fp16 cast (SWDGE), laid out
    partition-contiguous: nat[p, j, u] = ctx[16p+j, u] (fast descriptors).
    All t-indexing downstream inherits this scrambled order consistently
    (softmax is permutation-invariant; ctxT rows and replicated-attn rows
    pair up in the same order), so correctness is unaffected.
  - ctxT produced by PE transposes (identity matmul, ~60-90ns per 128x128
    f16 tile) packed into f16 PSUM tiles, unpacked to SBUF on the Scalar
    engine (keeps the Vector engine off the transpose critical path).
    Transposes are interleaved with score matmuls to keep the PE HAM warm.
  - scores matmul: Wa stationary, ctxT moving -> S^T chunks in PSUM;
    tanh on ACT with per-partition bias = (q + ba_w)^T
  - Va dot via matmul (lhsT = Va chunk [128,1]) -> scores [1, t] in PSUM
  - exp on ACT with fused accum_out -> softmax normalizer (no max-subtract
    needed: |score| <= ||Va||_1 ~ 8, exp stays in fp32 range)
  - attn replicated across partitions (gpsimd partition_broadcast); ctx_vec
    computed on the Vector engine via scalar_tensor_tensor with accum_out
    (fused multiply+reduce over ctxT; NOTE tensor_tensor_reduce hangs the
    device in this environment, scalar_tensor_tensor works)
  - gate math per 4-batch group on partitions 0..3 (SBUF compute APs may
    only start at partitions {0,32,64,96}); group-post emission is delayed
    behind the next batch's work to avoid PE head-of-line blocking
"""

import sys

if "/opt/trn_rl_repo" not in sys.path:
    sys.path.insert(0, "/opt/trn_rl_repo")

import numpy as np

import concourse.bass as bass
import concourse.mybir as mybir
import concourse.tile as tile
from concourse import bacc

F32 = mybir.dt.float32
F16 = mybir.dt.float16
AF = mybir.ActivationFunctionType
OP = mybir.AluOpType

B = 64          # total batch
T = 2048        # context length
U = 512         # units
EMB = 256
NCORES = 8
BPC = B // NCORES   # batches per core
KU = U // 128       # 4 k-chunks over units
TC = T // 128       # 16 t-chunks


def _build_program():
    nc = bacc.Bacc("TRN2", target_bir_lowering=False, debug=False, num_devices=NCORES)

    # ---- DRAM I/O ----
    ctx_d = nc.dram_tensor("ctx", [BPC, T, U], F32, kind="ExternalInput").ap()
    inp_d = nc.dram_tensor("inp", [BPC, EMB], F32, kind="ExternalInput").ap()
    h0_d = nc.dram_tensor("h0", [BPC, U], F32, kind="ExternalInput").ap()

    wa_d = nc.dram_tensor("wa16", [U, U], F16, kind="ExternalInput").ap()
    wa8_d = nc.dram_tensor("wa8dr", [128, 2, 2, KU, 128], mybir.dt.float8e4,
                           kind="ExternalInput").ap()
    ua_d = nc.dram_tensor("ua16", [U, U], F16, kind="ExternalInput").ap()
    wi_d = nc.dram_tensor("wi16", [EMB, U], F16, kind="ExternalInput").ap()
    kern_d = nc.dram_tensor("kern16", [U, 3 * U], F16, kind="ExternalInput").ap()
    rec_d = nc.dram_tensor("rec16", [U, 3 * U], F16, kind="ExternalInput").ap()
    attk_d = nc.dram_tensor("attk16", [U, 3 * U], F16, kind="ExternalInput").ap()
    wo_d = nc.dram_tensor("wo16", [U, U], F16, kind="ExternalInput").ap()
    vat_d = nc.dram_tensor("va_t", [128, KU], F16, kind="ExternalInput").ap()
    id_d = nc.dram_tensor("ident16", [128, 128], F16, kind="ExternalInput").ap()

    bi_d = nc.dram_tensor("bi", [U], F32, kind="ExternalInput").ap()
    bg_d = nc.dram_tensor("biasg", [3 * U], F32, kind="ExternalInput").ap()
    bau_d = nc.dram_tensor("ba_u", [U], F32, kind="ExternalInput").ap()
    bawt_d = nc.dram_tensor("ba_wt8", [128, KU, BPC], F32, kind="ExternalInput").ap()
    bav_d = nc.dram_tensor("ba_v1", [1, 1], F32, kind="ExternalInput").ap()
    bo_d = nc.dram_tensor("bo", [U], F32, kind="ExternalInput").ap()

    out_d = nc.dram_tensor("out_o", [BPC, U], F32, kind="ExternalOutput").ap()
    h_d = nc.dram_tensor("h_o", [BPC, U], F32, kind="ExternalOutput").ap()

    with tile.TileContext(nc) as tc:
        _emit(nc, tc, locals())
    nc.compile()
    return nc


def _bcast_rows(ap_1d, rows, cols):
    """DMA source AP replicating a 1-D [cols] dram tensor across `rows` partitions."""
    return bass.AP(ap_1d.tensor, 0, [[0, rows], [1, cols]])


def _emit(nc, tc, d):
    ctx_d, inp_d, h0_d = d["ctx_d"], d["inp_d"], d["h0_d"]
    wa_d, ua_d, wi_d, kern_d = d["wa_d"], d["ua_d"], d["wi_d"], d["kern_d"]
    wa8_d = d["wa8_d"]
    rec_d, attk_d, wo_d, vat_d, id_d = (
        d["rec_d"], d["attk_d"], d["wo_d"], d["vat_d"], d["id_d"],
    )
    bi_d, bg_d, bau_d, bawt_d, bav_d, bo_d = (
        d["bi_d"], d["bg_d"], d["bau_d"], d["bawt_d"], d["bav_d"], d["bo_d"],
    )
    out_d, h_d = d["out_d"], d["h_d"]

    from contextlib import ExitStack

    es = ExitStack()
    wp = es.enter_context(tc.tile_pool(name="weights", bufs=1))
    gp = es.enter_context(tc.tile_pool(name="group", bufs=2))
    natp = es.enter_context(tc.tile_pool(name="nat", bufs=4))
    natTp = es.enter_context(tc.tile_pool(name="natT", bufs=2))
    thp = es.enter_context(tc.tile_pool(name="tanh", bufs=6))
    # PSUM budget: 8 banks total.
    # ps_S: [128,1024]f32 = 2 banks x 2 bufs = 4;  ps_sc: [1,1024] = 2 banks x 1;
    # ps_sm: <=1 bank x 2 bufs = 2.
    pS = es.enter_context(tc.tile_pool(name="psS", bufs=2, space="PSUM"))
    pSC = es.enter_context(tc.tile_pool(name="psSC", bufs=1, space="PSUM"))
    pp = es.enter_context(tc.tile_pool(name="psSM", bufs=1, space="PSUM"))
    pT = es.enter_context(tc.tile_pool(name="psT", bufs=1, space="PSUM"))

    # ---- load weights (one-time) ----
    def load_kxm(pool, dram, rows, cols, tag):
        t = pool.tile([128, rows // 128, cols], F16, tag=tag, name=tag)
        src = bass.AP(dram.tensor, 0, [[cols, 128], [128 * cols, rows // 128], [1, cols]])
        nc.sync.dma_start(out=t, in_=src)
        return t

    id_sb = wp.tile([128, 128], F16)
    nc.sync.dma_start(out=id_sb, in_=id_d)
    va_sb = wp.tile([128, KU], F16)
    nc.sync.dma_start(out=va_sb, in_=vat_d)
    wa8_sb = wp.tile([128, 2, 2, KU, 128], mybir.dt.float8e4)
    nc.sync.dma_start(out=wa8_sb, in_=wa8_d)

    # prefetch the first two context batches before the bulk weight loads
    natp = es.enter_context(tc.tile_pool(name="nat", bufs=2))
    nat_pre = {}
    for pb_ in range(2):
        t = natp.tile([128, TC, U], F16, tag="nat", name=f"natp{pb_}")
        nc.gpsimd.dma_start(out=t, in_=bass.AP(
            ctx_d.tensor, pb_ * T * U, [[TC * U, 128], [U, TC], [1, U]]))
        nat_pre[pb_] = t

    rec_sb = load_kxm(wp, rec_d, U, 3 * U, "recw")
    attk_sb = load_kxm(wp, attk_d, U, 3 * U, "attkw")
    wo_sb = load_kxm(wp, wo_d, U, U, "wow")

    bo8 = wp.tile([BPC, U], F32)
    nc.sync.dma_start(out=bo8, in_=_bcast_rows(bo_d, BPC, U))
    bawt8 = wp.tile([128, KU, BPC], F32)
    nc.sync.dma_start(out=bawt8, in_=bawt_d)
    bav_sb = wp.tile([1, 1], F32)
    nc.sync.dma_start(out=bav_sb, in_=bav_d)

    h032 = wp.tile([BPC, U], F32)
    nc.sync.dma_start(out=h032, in_=h0_d)

    # ---- helpers ----
    def transpose_to(dst_f16, src, nrow, chunks):
        """PE-transpose src [nrow, chunks*128] f16 -> dst [128, chunks*nrow] f16.
        dst column layout: chunk-major, row-minor."""
        pm = pp.tile([128, chunks * nrow], F16, tag="sm")
        for c in range(chunks):
            nc.tensor.transpose(
                pm[:, c * nrow:(c + 1) * nrow],
                src[0:nrow, c * 128:(c + 1) * 128],
                id_sb[0:nrow, 0:nrow],
            )
        nc.vector.tensor_copy(dst_f16, pm[:, 0:chunks * nrow])

    # ---- phase 0: small dense matmuls for all 8 local batches ----
    # Everything here (incl. kernel/Ua/Wi weights) is used once, so it lives in
    # a scoped pool whose SBUF is reclaimed before the streaming phase.
    qb = wp.tile([128, KU, BPC], F32)
    xg = wp.tile([BPC, 3 * U], F32)
    reczr = wp.tile([BPC, 2 * U], F32)
    out_sb = wp.tile([BPC, U], F32)
    h_sb = wp.tile([BPC, U], F32)

    with tc.tile_pool(name="phase0", bufs=1) as p0:
        ua_sb = load_kxm(p0, ua_d, U, U, "uaw")
        wi_sb = load_kxm(p0, wi_d, EMB, U, "wiw")
        kern_sb = load_kxm(p0, kern_d, U, 3 * U, "kernw")

        bi8 = p0.tile([BPC, U], F32)
        nc.sync.dma_start(out=bi8, in_=_bcast_rows(bi_d, BPC, U))
        bg8 = p0.tile([BPC, 3 * U], F32)
        nc.sync.dma_start(out=bg8, in_=_bcast_rows(bg_d, BPC, 3 * U))
        bau8 = p0.tile([BPC, U], F32)
        nc.sync.dma_start(out=bau8, in_=_bcast_rows(bau_d, BPC, U))
        inp16 = p0.tile([BPC, EMB], F16)
        nc.gpsimd.dma_start(out=inp16, in_=inp_d)
        h016 = p0.tile([BPC, U], F16)
        nc.gpsimd.dma_start(out=h016, in_=h0_d)

        inT = p0.tile([128, 2 * BPC], F16)
        transpose_to(inT, inp16, BPC, 2)
        hT = p0.tile([128, KU * BPC], F16)
        transpose_to(hT, h016, BPC, KU)

        # x = inputs @ Wi + bi
        px = pp.tile([BPC, U], F32, tag="sm")
        for c in range(2):
            nc.tensor.matmul(px, inT[:, c * BPC:(c + 1) * BPC], wi_sb[:, c, :],
                             start=(c == 0), stop=(c == 1))
        x16 = p0.tile([BPC, U], F16)
        nc.vector.tensor_add(x16, px, bi8)
        xT = p0.tile([128, KU * BPC], F16)
        transpose_to(xT, x16, BPC, KU)

        # xg = x @ kernel + bias  (kept resident, fp32)
        for n in range(3):
            pg = pp.tile([BPC, U], F32, tag="sm")
            for c in range(KU):
                nc.tensor.matmul(pg, xT[:, c * BPC:(c + 1) * BPC],
                                 kern_sb[:, c, n * U:(n + 1) * U],
                                 start=(c == 0), stop=(c == KU - 1))
            nc.vector.tensor_add(xg[:, n * U:(n + 1) * U], pg,
                                 bg8[:, n * U:(n + 1) * U])

        # q = h @ Ua + ba_u ; transposed, +ba_w -> tanh bias  [128, KU, BPC]
        pq = pp.tile([BPC, U], F32, tag="sm")
        for c in range(KU):
            nc.tensor.matmul(pq, hT[:, c * BPC:(c + 1) * BPC], ua_sb[:, c, :],
                             start=(c == 0), stop=(c == KU - 1))
        q16 = p0.tile([BPC, U], F16)
        nc.vector.tensor_add(q16, pq, bau8)
        pmq = pp.tile([128, KU * BPC], F16, tag="sm")
        for c in range(KU):
            nc.tensor.transpose(pmq[:, c * BPC:(c + 1) * BPC],
                                q16[0:BPC, c * 128:(c + 1) * 128],
                                id_sb[0:BPC, 0:BPC])
        for c in range(KU):
            nc.vector.tensor_add(qb[:, c, :], pmq[:, c * BPC:(c + 1) * BPC],
                                 bawt8[:, c, :])

        # rec_zr = h @ recurrent[:, :2U]  (resident fp32)
        for n in range(2):
            pr = pp.tile([BPC, U], F32, tag="sm")
            for c in range(KU):
                nc.tensor.matmul(pr, hT[:, c * BPC:(c + 1) * BPC],
                                 rec_sb[:, c, n * U:(n + 1) * U],
                                 start=(c == 0), stop=(c == KU - 1))
            nc.vector.tensor_copy(reczr[:, n * U:(n + 1) * U], pr)

    # ---- streaming over batches, groups of 4 ----
    nats = {}
    expsb = None
    zp = None
    for b in range(BPC):
        gi = b % 4
        grp = b // 4
        if gi == 0:
            expsb = gp.tile([4, T], F16, tag="expsb")
            zp = gp.tile([4, 2], F32, tag="zp")

        # load natural ctx (fp32 -> fp16 cast), [128 tp, TC, U]
        nat = natp.tile([128, TC, U], F16, tag="nat")
        src = bass.AP(ctx_d.tensor, b * T * U, [[U, 128], [128 * U, TC], [1, U]])
        nc.gpsimd.dma_start(out=nat, in_=src)
        nats[b] = nat

        # scores, per t-half of 1024 (ctxT transposed just-in-time per half)
        for th in range(2):
            # xbar transpose -> ctxT half [128 up, KU, 1024]
            natT = natTp.tile([128, KU, 1024], mybir.dt.float8e4, tag="natT")
            for tci in range(8):
                tcg = th * 8 + tci
                for uc in range(KU):
                    nc.sync.dma_start(
                        out=natT[:, uc, tci * 128:(tci + 1) * 128],
                        in_=nat[:, tcg, uc * 128:(uc + 1) * 128],
                        transpose=True,
                    )

            psc = pSC.tile([1, 1024], F32, tag="sc")
            th16s = []
            for m in range(KU):  # uo chunk
                ps = pS.tile([128, 1024], F32, tag="S")
                for half in range(2):
                    for k in range(KU):
                        nc.tensor.matmul(
                            ps[:, half * 512:(half + 1) * 512],
                            wa_sb[:, k, m * 128:(m + 1) * 128],
                            natT[:, k, half * 512:(half + 1) * 512],
                            start=(k == 0), stop=(k == KU - 1),
                        )
                th16 = thp.tile([128, 1024], F16, tag="th")
                nc.scalar.activation(th16, ps, AF.Tanh, bias=qb[:, m, b:b + 1])
                th16s.append(th16)
            # Va dot: consecutive accumulation groups per psum bank
            for half in range(2):
                for m in range(KU):
                    nc.tensor.matmul(
                        psc[0:1, half * 512:(half + 1) * 512],
                        va_sb[:, m:m + 1],
                        th16s[m][:, half * 512:(half + 1) * 512],
                        start=(m == 0), stop=(m == KU - 1),
                    )
            nc.scalar.activation(
                expsb[gi:gi + 1, th * 1024:(th + 1) * 1024], psc, AF.Exp,
                bias=bav_sb[0:1, 0:1], accum_out=zp[gi:gi + 1, th:th + 1],
            )

        if gi == 3:
            pending.append((grp, cvT16))
        if len(pending) and b % 4 == 1 and b > 1:
            g0, cv0 = pending.pop(0)
            emit_group_post(g0, cv0, h032g[g0], xgg[g0])

    while pending:
        g0, cv0 = pending.pop(0)
        emit_group_post(g0, cv0, h032g[g0], xgg[g0])

    es.close()


_PROGRAM = None


def _get_program():
    global _PROGRAM
    if _PROGRAM is None:
        _PROGRAM = _build_program()
    return _PROGRAM


def make_in_maps(inputs, h_tm1, context, Wi, bi, kernel, recurrent_kernel,
                 attention_kernel, bias, Wa, ba_w, Ua, ba_u, Va, ba_v, Wo, bo):
    f32 = lambda x: np.ascontiguousarray(np.asarray(x, dtype=np.float32))
    f16 = lambda x: np.ascontiguousarray(np.asarray(x, dtype=np.float32).astype(np.float16))

    context = f32(context)
    inputs = f32(inputs)
    h_tm1 = f32(h_tm1)

    wa32 = np.asarray(Wa, np.float32) * 16.0
    f8 = mybir.dt.np(mybir.dt.float8e4)
    wa8dr = np.zeros((128, 2, 2, KU, 128), np.float32)
    for c in range(2):
        for i in range(2):
            for mc in range(KU):
                # lhsT[p, i, m] = Wa'[c*256 + i*128 + p, mc*128 + m]
                wa8dr[:, c, i, mc, :] = wa32[c * 256 + i * 128: c * 256 + (i + 1) * 128,
                                             mc * 128:(mc + 1) * 128]
    shared = {
        "wa8dr": np.ascontiguousarray(wa8dr.astype(f8)),
        "wa16": f16(Wa), "ua16": f16(Ua), "wi16": f16(Wi),
        "kern16": f16(kernel), "rec16": f16(recurrent_kernel),
        "attk16": f16(attention_kernel), "wo16": f16(Wo),
        "va_t": np.ascontiguousarray(
            np.asarray(Va, np.float32).reshape(KU, 128).T.astype(np.float16)),
        "ident16": np.eye(128, dtype=np.float16),
        "bi": f32(bi), "biasg": f32(bias), "ba_u": f32(ba_u),
        "ba_wt8": np.ascontiguousarray(np.repeat(
            np.asarray(ba_w, np.float32).reshape(KU, 128).T[:, :, None], BPC, axis=2)),
        "ba_v1": f32(ba_v).reshape(1, 1),
        "bo": f32(bo),
    }
    in_maps = []
    for i in range(NCORES):
        s = slice(i * BPC, (i + 1) * BPC)
        in_maps.append({
            "ctx": context[s], "inp": inputs[s], "h0": h_tm1[s], **shared,
        })
    return in_maps


def kernel(**inputs):
    from concourse.bass_utils import run_bass_kernel_spmd

    nc = _get_program()
    in_maps = make_in_maps(**inputs)
    res = run_bass_kernel_spmd(nc, in_maps, list(range(NCORES)))
    out = np.concatenate([r["out_o"] for r in res.results], axis=0)
    h = np.concatenate([r["h_o"] for r in res.results], axis=0)
    return out.astype(np.float32), h.astype(np.float32)


if __name__ == "__main__":
    prog = _get_program()
    print("program built OK:", len(prog.m.functions[0].instructions) if hasattr(prog.m.functions[0], "instructions") else "?")
